# revision 39
# baseline (speedup 1.0000x reference)
"""Trainium2 Bass kernel for nn_Attention_68006512164916.

EVA-style vision attention block: qkv proj -> 2D rope (interleaved pairs)
-> SDPA (16 heads, d=64, seq 256) -> out proj. B=64, N=256, C=1024, fp32 I/O.

Strategy: data-parallel over batch across 8 NeuronCores (8 batches/core,
no collectives). Per core, everything is computed in bf16 on the
TensorEngine with fp32 PSUM accumulation:

  - host: x is transposed/cast to xT [C, B_loc*N] bf16 so the QKV matmul
    needs no on-device transpose of x; qkv_w rows for q/k are permuted
    (per-head d-interleave -> [evens|odds]) so rope becomes half-block
    free-axis ops; q rows pre-scaled by D^-0.5; proj_w pre-transposed.
  - qkv = xT.T @ wT  (option A layout [n, o]) -> PSUM -> bf16 SBUF
  - rope applied as 6 DVE tensor_tensor ops per [128,1024] tile
  - q,k transposed per 128x128 block on the TensorEngine (identity mm)
    into qT/kT [d, n] layout for attention
  - S^T[j,i] = kT.T @ qT (K=64), exp on ScalarE (no max subtraction:
    |scores| <= ~6 for this input distribution).
  - fused-sums PV: v is stored interleaved per head as [v_h | ones64]
    ([128, 16, 128] SBUF tiles, ones lanes memset by GpSimd), so each
    PV matmul emits O^T rows 0..63 AND the softmax denominator
    replicated across rows 64..127 of the same [128, 256] PSUM tile —
    the row-sum selector matmuls and the reciprocal DRAM-bounce
    broadcast of the old design are gone. Normalization is local:
    reciprocal_approx_fast on rows 64..127 then one tensor_tensor mul
    into the bf16 oT tile.
  - y = O^T.T @ pwT + b, streamed out as fp32.
"""

import sys

if "/opt/trn_rl_repo" not in sys.path:
    sys.path.insert(0, "/opt/trn_rl_repo")

import numpy as np
import ml_dtypes

import concourse.bacc as bacc
import concourse.mybir as mybir
import concourse.tile as tile
from concourse.bass_utils import run_bass_kernel_spmd
from concourse.masks import make_identity

f32 = mybir.dt.float32
bf16 = mybir.dt.bfloat16

N_CORES = 8
B, N, C = 64, 256, 1024
H, D = 16, 64
B_LOC = B // N_CORES          # 8 batches per core
NT = B_LOC * N                # 2048 token rows per core
HW = 16
THETA = 10000.0

_cache = {}


def _rope_tables():
    hd = D // 2  # 32
    inv_freq = 1.0 / (THETA ** (np.arange(0, hd, 2, dtype=np.float32) / hd))
    t = np.arange(HW, dtype=np.float32)
    f = np.einsum("i,j->ij", t, inv_freq)          # (16, 16)
    f = np.repeat(f, 2, axis=-1)                   # (16, 32)
    fx = np.broadcast_to(f[:, None, :], (HW, HW, hd))
    fy = np.broadcast_to(f[None, :, :], (HW, HW, hd))
    F = np.concatenate([fx, fy], axis=-1).reshape(N, D)  # (256, 64)
    cosH = np.cos(F[:, 0::2])                      # (256, 32)
    sinH = np.sin(F[:, 0::2])
    return cosH.astype(np.float32), sinH.astype(np.float32)


def _build(with_bias=True):
    key = ("nc", with_bias)
    if key in _cache:
        return _cache[key]

    nc = bacc.Bacc("TRN2", target_bir_lowering=False, debug=False,
                   num_devices=N_CORES)

    xT_d = nc.dram_tensor("xT", [C, NT], bf16, kind="ExternalInput")
    w_d = nc.dram_tensor("wT", [C, 3 * C], bf16, kind="ExternalInput")
    pw_d = nc.dram_tensor("pwT", [C, C], bf16, kind="ExternalInput")
    pb_d = nc.dram_tensor("pb", [1, C], f32, kind="ExternalInput")
    cos_d = nc.dram_tensor("cosH", [N, 32], bf16, kind="ExternalInput")
    sin_d = nc.dram_tensor("sinH", [N, 32], bf16, kind="ExternalInput")
    out_d = nc.dram_tensor("out", [NT, C], f32, kind="ExternalOutput")

    Exp = mybir.ActivationFunctionType.Exp
    MUL = mybir.AluOpType.mult
    ADD = mybir.AluOpType.add
    SUB = mybir.AluOpType.subtract

    from contextlib import ExitStack
    with tile.TileContext(nc) as tc:
        with ExitStack() as ctx:
            const = ctx.enter_context(tc.tile_pool(name="const", bufs=1))
            xg_p = ctx.enter_context(tc.tile_pool(name="xg", bufs=2))
            qkraw_p = ctx.enter_context(tc.tile_pool(name="qkraw", bufs=2))
            tmp_p = ctx.enter_context(tc.tile_pool(name="tmp", bufs=4))
            qkrot_p = ctx.enter_context(tc.tile_pool(name="qkrot", bufs=9))
            v_p = ctx.enter_context(tc.tile_pool(name="vg", bufs=2))
            qkT_p = ctx.enter_context(tc.tile_pool(name="qkT", bufs=2))
            pT_p = ctx.enter_context(tc.tile_pool(name="pT", bufs=5))
            oT_p = ctx.enter_context(tc.tile_pool(name="oT", bufs=2))
            rsb_p = ctx.enter_context(tc.tile_pool(name="rsb", bufs=5))
            y_p = ctx.enter_context(tc.tile_pool(name="y", bufs=2))
            # PSUM budget (8 banks): big 2-bank tiles x2 (qkv pairs +
            # 2-head score tiles) + proj 1-bank x2 + PV pairs 1-bank x2
            big_p = ctx.enter_context(tc.tile_pool(name="big", bufs=2, space="PSUM"))
            psmm_p = ctx.enter_context(tc.tile_pool(name="psmm", bufs=2, space="PSUM"))
            pso_p = ctx.enter_context(tc.tile_pool(name="pso", bufs=2, space="PSUM"))

            # ---- constants ----
            # split the 6MB weight load into per-chunk DMAs spread across
            # the per-engine DMA queues so they run in parallel and the
            # first qkv matmuls can start as soon as chunk 0 lands
            w_sb = [[const.tile([128, C], bf16, name=f"w_sb{fp}_{cc}")
                     for cc in range(8)] for fp in range(3)]
            w_r = w_d.ap().rearrange("(co ci) (fp o) -> ci co fp o",
                                     ci=128, fp=3)
            cos_sb = const.tile([128, 2, 32], bf16)
            nc.sync.dma_start(cos_sb[:], cos_d.ap().rearrange(
                "(nt p) t -> p nt t", p=128))
            sin_sb = const.tile([128, 2, 32], bf16)
            nc.sync.dma_start(sin_sb[:], sin_d.ap().rearrange(
                "(nt p) t -> p nt t", p=128))
            ident = const.tile([128, 128], bf16)
            make_identity(nc, ident)
            # pw/pb are allocated here but loaded after group 0's qkv
            # emission (queue FIFO keeps them out of the critical startup
            # bandwidth window; they are first read ~60us in)
            pw_sb = [const.tile([128, C], bf16, name=f"pw_sb{cc}")
                     for cc in range(8)]
            pb_bc = const.tile([128, C], f32)
            pw_r = pw_d.ap().rearrange("(co ci) o -> ci co o", ci=128)

            xT_r = xT_d.ap().rearrange("(co ci) n -> ci co n", ci=128)
            pending = []

            # keep the Scalar (ACT) queue clear of DMA issues — descriptor
            # generation occupies the issuing engine's FIFO and delays
            # exp/evictions queued behind it
            xfer_q = [nc.sync, nc.gpsimd]

            def load_xg(g):
                xg = [xg_p.tile([128, 512], bf16, tag=f"xg{cc}",
                                name=f"xg_{g}_{cc}") for cc in range(8)]
                for cc in range(8):
                    xfer_q[cc % 2].dma_start(
                        xg[cc][:], xT_r[:, cc, g * 512:(g + 1) * 512])
                return xg

            xg_next = None
            for g in range(4):          # group = 2 batches (512 token cols)
                if g == 0:
                    # startup-critical ordering: queues are FIFO, so land
                    # chunks in consumption order (xg chunk 0 first, then q
                    # weights (fp0), k (fp1), then v (fp2) on the gpsimd
                    # queue behind xg). pw/pb wait until after the qkv
                    # emission. Later groups' xg loads are prefetched
                    # during the previous group's attention phase.
                    xg = [xg_p.tile([128, 512], bf16, tag=f"xg{cc}",
                                    name=f"xg_0_{cc}") for cc in range(8)]
                    nc.gpsimd.dma_start(xg[0][:], xT_r[:, 0, 0:512])
                    for cc in range(1, 8):
                        xfer_q[cc % 2].dma_start(xg[cc][:],
                                                 xT_r[:, cc, 0:512])
                    for fp in range(3):
                        for cc in range(8):
                            xfer_q[cc % 2].dma_start(
                                w_sb[fp][cc][:], w_r[:, cc, fp, :])
                else:
                    xg = xg_next

                # v is stored head-interleaved with ones lanes for the
                # fused-sums PV: slot h = [v_h | ones] (even h) or
                # [ones | v_h] (odd h) so O rows and sum rows alternate
                # PSUM halves per head parity and every normalize op is
                # 64-partition aligned.
                v_g = [v_p.tile([128, 16, 128], bf16, tag=f"vg{ns}",
                                name=f"v_{g}_{ns}") for ns in range(4)]
                for ns in range(4):
                    v4 = v_g[ns][:].rearrange("p (pr q) c -> p pr q c", q=2)
                    nc.gpsimd.memset(v4[:, :, 0, 64:128], 1.0)
                    nc.gpsimd.memset(v4[:, :, 1, 0:64], 1.0)
                # q and k transposed blocks for head-pair p8 share one
                # [128, 2, 512] tile (q at half 0, k at half 1) so each
                # pair needs a single PSUM bank and a single eviction
                qkT_g = [qkT_p.tile([128, 2, 512], bf16, tag=f"qkT{p8}",
                                    name=f"qkT_{g}_{p8}") for p8 in range(8)]
                rot_tiles = {}

                # ---- qkv matmuls + rope ----
                # the two 512-wide halves of each 1024-col output share the
                # same stationary lhsT per k-chunk, letting walrus reuse the
                # loaded weights between consecutive matmuls
                for fp in range(3):     # 0: q, 1: k, 2: v (1024 cols each)
                    for ns in range(4):
                        if fp < 2:
                            raw = qkraw_p.tile([128, H, D], bf16, tag="qkraw")
                            rawf = raw[:].rearrange("p h d -> p (h d)")
                        pss = big_p.tile([128, 1024], f32, tag="big",
                                         name=f"ps_{fp}_{ns}")
                        for cc in range(8):
                            for half in range(2):
                                nc.tensor.matmul(
                                    pss[:, half * 512:(half + 1) * 512],
                                    lhsT=xg[cc][:, ns * 128:(ns + 1) * 128],
                                    rhs=w_sb[fp][cc][:, half * 512:(half + 1) * 512],
                                    start=(cc == 0), stop=(cc == 7))
                        if fp == 2:
                            # heads half*8..half*8+7: even slots keep v in
                            # cols 0:64, odd slots in cols 64:128
                            v4 = v_g[ns][:].rearrange(
                                "p (pr q) c -> p pr q c", q=2)
                            s4 = pss[:].rearrange(
                                "p (pr q c) -> p pr q c", q=2, c=64)
                            nc.any.tensor_copy(
                                out=v4[:, :, 0, 0:64], in_=s4[:, :, 0, :])
                            nc.any.tensor_copy(
                                out=v4[:, :, 1, 64:128], in_=s4[:, :, 1, :])
                        else:
                            nc.any.tensor_copy(out=rawf[:], in_=pss[:])
                        if fp == 2:
                            continue
                        # rope: evens = raw[:,:,0:32], odds = raw[:,:,32:64]
                        nt = ns % 2
                        cos = cos_sb[:, nt, None, :].to_broadcast((128, H, 32))
                        sin = sin_sb[:, nt, None, :].to_broadcast((128, H, 32))
                        qe = raw[:, :, 0:32]
                        qo = raw[:, :, 32:64]
                        t1 = tmp_p.tile([128, H, 32], bf16, tag="tmp")
                        t2 = tmp_p.tile([128, H, 32], bf16, tag="tmp")
                        nc.vector.tensor_tensor(out=t1[:], in0=qe, in1=cos, op=MUL)
                        nc.vector.tensor_tensor(out=t2[:], in0=qo, in1=sin, op=MUL)
                        rot = qkrot_p.tile([128, H, D], bf16, tag="rot")
                        nc.vector.tensor_tensor(out=rot[:, :, 0:32],
                                                in0=t1[:], in1=t2[:], op=SUB)
                        t3 = tmp_p.tile([128, H, 32], bf16, tag="tmp")
                        t4 = tmp_p.tile([128, H, 32], bf16, tag="tmp")
                        nc.vector.tensor_tensor(out=t3[:], in0=qo, in1=cos, op=MUL)
                        nc.vector.tensor_tensor(out=t4[:], in0=qe, in1=sin, op=MUL)
                        nc.vector.tensor_tensor(out=rot[:, :, 32:64],
                                                in0=t3[:], in1=t4[:], op=ADD)
                        rot_tiles[(fp, ns)] = rot

                # ---- transpose q,k into [d, n] layout (PE identity mm) ----
                # emitted in score-consumption order (pair 0, 1, ...) so
                # batch 0's first scores never wait on a late eviction
                for p8 in range(8):
                    fcol = p8 * 128
                    pst = psmm_p.tile([128, 2, 512], bf16, tag="mm",
                                      name=f"pst_{g}_{p8}")
                    for half in range(2):   # 0: q block, 1: k block
                        for ns in range(4):
                            rot = rot_tiles[(half, ns)]
                            nc.tensor.transpose(
                                pst[:, half, ns * 128:(ns + 1) * 128],
                                rot[:].rearrange("p h d -> p (h d)")[:, fcol:fcol + 128],
                                ident)
                    nc.any.tensor_copy(out=qkT_g[p8][:], in_=pst[:])
                if g == 0:
                    for cc in range(8):
                        xfer_q[cc % 2].dma_start(
                            pw_sb[cc][:], pw_r[:, cc, :])
                    nc.sync.dma_start(pb_bc[:],
                                      pb_d.ap().to_broadcast((128, C)))
                # prefetch next group's x chunks so the group-boundary qkv
                # matmuls never wait on HBM
                xg_next = load_xg(g + 1) if g < 3 else None

                # ---- attention per batch ----
                # per-head pipeline: scores (PE) -> exp (ScalarE) ->
                # fused-sums PV (PE, O rows + replicated denominator in one
                # [128, 256] PSUM region) -> approx-reciprocal + normalize
                # mul (DVE, all local, no DMA). The previous batch's output
                # projection is spread through this batch's head loop in 4
                # chunks so its PSUM-accumulation tails overlap scores.
                for bb in range(2):
                    b_loc = 2 * g + bb
                    oT_b = oT_p.tile([128, 8, 256], bf16, tag="oT")
                    pTs = {}
                    # head-pair loop, software-pipelined by two pairs:
                    # scores of both heads of pair `m` land in one 2-bank
                    # PSUM tile so a single exp covers them, and the PV +
                    # normalize of pair `m-2` are emitted after pair `m`'s
                    # scores so the in-order PE never stalls scores behind
                    # a PV whose exp has not drained from the ACT queue.
                    for m in range(10):
                        if m < 8:
                            ps_s = big_p.tile([128, 4, 256], f32, tag="big")
                            for h2 in range(2):
                                h = 2 * m + h2
                                qk, qrow = qkT_g[h // 2], (h % 2) * 64
                                for jc in range(2):
                                    nc.tensor.matmul(
                                        ps_s[:, h2 * 2 + jc, :],
                                        lhsT=qk[qrow:qrow + 64, 1,
                                                bb * 256 + jc * 128:bb * 256 + jc * 128 + 128],
                                        rhs=qk[qrow:qrow + 64, 0,
                                               bb * 256:bb * 256 + 256],
                                        start=True, stop=True)
                            pT = pT_p.tile([128, 4, 256], bf16, tag="pT")
                            nc.scalar.activation(pT[:], ps_s[:], Exp)
                            pTs[m] = pT
                        if m >= 2:
                            mp = m - 2
                            pT = pTs.pop(mp)
                            ps_pair = pso_p.tile([128, 512], f32, tag="o")
                            for h2 in range(2):
                                h = 2 * mp + h2
                                po = ps_pair[:, h2 * 256:h2 * 256 + 256]
                                for jc in range(2):
                                    nc.tensor.matmul(
                                        po,
                                        lhsT=v_g[bb * 2 + jc][:].rearrange(
                                            "p s c -> p (s c)")[:, h * 128:(h + 1) * 128],
                                        rhs=pT[:, h2 * 2 + jc, :],
                                        start=(jc == 0), stop=(jc == 1))
                            # normalize the pair: gather the two heads'
                            # replicated sums (window-aligned copies, one
                            # on DVE and one on ScalarE to balance engine
                            # load; the custom-DVE approx reciprocal only
                            # works at base partition 0 full-window), one
                            # reciprocal, two muls with cross-window in1
                            # reads (verified OK on hw).
                            ssb = rsb_p.tile([128, 256], f32, tag="ssb")
                            if mp % 2 == 0:
                                nc.vector.tensor_copy(
                                    out=ssb[64:128, :],
                                    in_=ps_pair[64:128, 0:256])
                                nc.scalar.copy(out=ssb[0:64, :],
                                               in_=ps_pair[0:64, 256:512])
                            else:
                                nc.scalar.copy(out=ssb[64:128, :],
                                               in_=ps_pair[64:128, 0:256])
                                nc.vector.tensor_copy(
                                    out=ssb[0:64, :],
                                    in_=ps_pair[0:64, 256:512])
                            rsb = rsb_p.tile([128, 256], f32, tag="rsb")
                            nc.vector.reciprocal_approx_fast(rsb[:], ssb[:])
                            nc.vector.tensor_tensor(
                                out=oT_b[0:64, mp, :],
                                in0=ps_pair[0:64, 0:256],
                                in1=rsb[64:128, :], op=MUL)
                            nc.vector.tensor_tensor(
                                out=oT_b[64:128, mp, :],
                                in0=ps_pair[64:128, 256:512],
                                in1=rsb[0:64, :], op=MUL)
                        if m % 2 == 1 and pending:
                            pending.pop(0)()

                    # ---- output projection (deferred, 4 chunks) ----
                    def make_proj(b_loc, oT_b):
                        chunks = []
                        for nt2 in range(2):
                          for oc in range(2):
                            def chunk(nt2=nt2, oc=oc, b_loc=b_loc, oT_b=oT_b):
                                ps_p = psmm_p.tile([128, 2, 256], f32,
                                                   tag="mm")
                                ps_pv = ps_p[:].rearrange("p a b -> p (a b)")
                                for cc in range(8):
                                    nc.tensor.matmul(
                                        ps_pv,
                                        lhsT=oT_b[:, cc,
                                                  nt2 * 128:(nt2 + 1) * 128],
                                        rhs=pw_sb[cc][:,
                                                      oc * 512:(oc + 1) * 512],
                                        start=(cc == 0), stop=(cc == 7))
                                y_sb = y_p.tile([128, 512], f32, tag="y")
                                if with_bias:
                                    nc.vector.tensor_tensor(
                                        out=y_sb[:], in0=ps_pv,
                                        in1=pb_bc[:, oc * 512:(oc + 1) * 512],
                                        op=ADD)
                                elif oc == 0:
                                    nc.scalar.copy(out=y_sb[:], in_=ps_pv)
                                else:
                                    nc.vector.tensor_copy(out=y_sb[:],
                                                          in_=ps_pv)
                                row0 = b_loc * 256 + nt2 * 128
                                nc.sync.dma_start(
                                    out_d.ap()[row0:row0 + 128,
                                               oc * 512:(oc + 1) * 512],
                                    y_sb[:])
                            chunks.append(chunk)
                        return chunks
                    pending.extend(make_proj(b_loc, oT_b))

            while pending:
                pending.pop(0)()

    nc.compile()
    _cache[key] = nc
    return nc


def _prep_inputs(x, qkv_w, proj_w, proj_b):
    perm = np.concatenate([np.arange(0, D, 2), np.arange(1, D, 2)])  # evens|odds
    head_perm = (np.arange(H)[:, None] * D + perm[None, :]).reshape(-1)
    wq = qkv_w[:C][head_perm] * np.float32(D ** -0.5)
    wk = qkv_w[C:2 * C][head_perm]
    wv = qkv_w[2 * C:]
    wT = np.ascontiguousarray(
        np.concatenate([wq, wk, wv], 0).T).astype(ml_dtypes.bfloat16)
    pwT = np.ascontiguousarray(proj_w.T).astype(ml_dtypes.bfloat16)
    pb = np.ascontiguousarray(proj_b.reshape(1, C)).astype(np.float32)
    cosH, sinH = _rope_tables()
    cosH = cosH.astype(ml_dtypes.bfloat16)
    sinH = sinH.astype(ml_dtypes.bfloat16)

    in_maps = []
    for c in range(N_CORES):
        xs = x[c * B_LOC:(c + 1) * B_LOC].reshape(NT, C)
        xT = np.ascontiguousarray(xs.T).astype(ml_dtypes.bfloat16)
        in_maps.append({"xT": xT, "wT": wT, "pwT": pwT, "pb": pb,
                        "cosH": cosH, "sinH": sinH})
    return in_maps


def _run(inputs, trace=False, **kw):
    nc = _build(with_bias=bool(np.any(inputs["proj_b"])))
    in_maps = _prep_inputs(inputs["x"], inputs["qkv_w"],
                           inputs["proj_w"], inputs["proj_b"])
    res = run_bass_kernel_spmd(nc, in_maps, core_ids=list(range(N_CORES)),
                               trace=trace, **kw)
    out = np.concatenate([res.results[c]["out"] for c in range(N_CORES)], 0)
    return out.reshape(B, N, C).astype(np.float32), res


def kernel(x, qkv_w, proj_w, proj_b):
    x = np.asarray(x, dtype=np.float32)
    qkv_w = np.asarray(qkv_w, dtype=np.float32)
    proj_w = np.asarray(proj_w, dtype=np.float32)
    proj_b = np.asarray(proj_b, dtype=np.float32)
    out, _ = _run({"x": x, "qkv_w": qkv_w, "proj_w": proj_w,
                   "proj_b": proj_b})
    return out



# revision 40
# speedup vs baseline: 1.0107x; 1.0107x over previous
"""Trainium2 Bass kernel for nn_Attention_68006512164916.

EVA-style vision attention block: qkv proj -> 2D rope (interleaved pairs)
-> SDPA (16 heads, d=64, seq 256) -> out proj. B=64, N=256, C=1024, fp32 I/O.

Strategy: data-parallel over batch across 8 NeuronCores (8 batches/core,
no collectives). Per core, everything is computed in bf16 on the
TensorEngine with fp32 PSUM accumulation:

  - host: x is transposed/cast to xT [C, B_loc*N] bf16 so the QKV matmul
    needs no on-device transpose of x; qkv_w rows for q/k are permuted
    (per-head d-interleave -> [evens|odds]) so rope becomes half-block
    free-axis ops; q rows pre-scaled by D^-0.5; proj_w pre-transposed.
  - qkv = xT.T @ wT  (option A layout [n, o]) -> PSUM -> bf16 SBUF
  - rope applied as 6 DVE tensor_tensor ops per [128,1024] tile
  - q,k transposed per 128x128 block on the TensorEngine (identity mm)
    into qT/kT [d, n] layout for attention
  - S^T[j,i] = kT.T @ qT (K=64), exp on ScalarE (no max subtraction:
    |scores| <= ~6 for this input distribution).
  - fused-sums PV: v is stored interleaved per head as [v_h | ones64]
    ([128, 16, 128] SBUF tiles, ones lanes memset by GpSimd), so each
    PV matmul emits O^T rows 0..63 AND the softmax denominator
    replicated across rows 64..127 of the same [128, 256] PSUM tile —
    the row-sum selector matmuls and the reciprocal DRAM-bounce
    broadcast of the old design are gone. Normalization is local:
    reciprocal_approx_fast on rows 64..127 then one tensor_tensor mul
    into the bf16 oT tile.
  - y = O^T.T @ pwT + b, streamed out as fp32.
"""

import sys

if "/opt/trn_rl_repo" not in sys.path:
    sys.path.insert(0, "/opt/trn_rl_repo")

import numpy as np
import ml_dtypes

import concourse.bacc as bacc
import concourse.mybir as mybir
import concourse.tile as tile
from concourse.bass_utils import run_bass_kernel_spmd
from concourse.masks import make_identity

f32 = mybir.dt.float32
bf16 = mybir.dt.bfloat16

N_CORES = 8
B, N, C = 64, 256, 1024
H, D = 16, 64
B_LOC = B // N_CORES          # 8 batches per core
NT = B_LOC * N                # 2048 token rows per core
HW = 16
THETA = 10000.0

_cache = {}


def _rope_tables():
    hd = D // 2  # 32
    inv_freq = 1.0 / (THETA ** (np.arange(0, hd, 2, dtype=np.float32) / hd))
    t = np.arange(HW, dtype=np.float32)
    f = np.einsum("i,j->ij", t, inv_freq)          # (16, 16)
    f = np.repeat(f, 2, axis=-1)                   # (16, 32)
    fx = np.broadcast_to(f[:, None, :], (HW, HW, hd))
    fy = np.broadcast_to(f[None, :, :], (HW, HW, hd))
    F = np.concatenate([fx, fy], axis=-1).reshape(N, D)  # (256, 64)
    cosH = np.cos(F[:, 0::2])                      # (256, 32)
    sinH = np.sin(F[:, 0::2])
    return cosH.astype(np.float32), sinH.astype(np.float32)


def _build(with_bias=True):
    key = ("nc", with_bias)
    if key in _cache:
        return _cache[key]

    nc = bacc.Bacc("TRN2", target_bir_lowering=False, debug=False,
                   num_devices=N_CORES)

    xT_d = nc.dram_tensor("xT", [C, NT], bf16, kind="ExternalInput")
    w_d = nc.dram_tensor("wT", [C, 3 * C], bf16, kind="ExternalInput")
    pw_d = nc.dram_tensor("pwT", [C, C], bf16, kind="ExternalInput")
    pb_d = nc.dram_tensor("pb", [1, C], f32, kind="ExternalInput")
    cos_d = nc.dram_tensor("cosH", [N, 32], bf16, kind="ExternalInput")
    sin_d = nc.dram_tensor("sinH", [N, 32], bf16, kind="ExternalInput")
    out_d = nc.dram_tensor("out", [NT, C], f32, kind="ExternalOutput")

    Exp = mybir.ActivationFunctionType.Exp
    MUL = mybir.AluOpType.mult
    ADD = mybir.AluOpType.add
    SUB = mybir.AluOpType.subtract

    from contextlib import ExitStack
    with tile.TileContext(nc) as tc:
        with ExitStack() as ctx:
            const = ctx.enter_context(tc.tile_pool(name="const", bufs=1))
            xg_p = ctx.enter_context(tc.tile_pool(name="xg", bufs=2))
            qkraw_p = ctx.enter_context(tc.tile_pool(name="qkraw", bufs=2))
            tmp_p = ctx.enter_context(tc.tile_pool(name="tmp", bufs=4))
            qkrot_p = ctx.enter_context(tc.tile_pool(name="qkrot", bufs=9))
            v_p = ctx.enter_context(tc.tile_pool(name="vg", bufs=2))
            qkT_p = ctx.enter_context(tc.tile_pool(name="qkT", bufs=2))
            pT_p = ctx.enter_context(tc.tile_pool(name="pT", bufs=5))
            oT_p = ctx.enter_context(tc.tile_pool(name="oT", bufs=2))
            rsb_p = ctx.enter_context(tc.tile_pool(name="rsb", bufs=5))
            y_p = ctx.enter_context(tc.tile_pool(name="y", bufs=2))
            # PSUM budget (8 banks): big 2-bank tiles x2 (qkv pairs +
            # 2-head score tiles) + proj 1-bank x2 + PV pairs 1-bank x2
            big_p = ctx.enter_context(tc.tile_pool(name="big", bufs=2, space="PSUM"))
            psmm_p = ctx.enter_context(tc.tile_pool(name="psmm", bufs=2, space="PSUM"))
            pso_p = ctx.enter_context(tc.tile_pool(name="pso", bufs=2, space="PSUM"))

            # ---- constants ----
            # split the 6MB weight load into per-chunk DMAs spread across
            # the per-engine DMA queues so they run in parallel and the
            # first qkv matmuls can start as soon as chunk 0 lands
            w_sb = [[const.tile([128, C], bf16, name=f"w_sb{fp}_{cc}")
                     for cc in range(8)] for fp in range(3)]
            w_r = w_d.ap().rearrange("(co ci) (fp o) -> ci co fp o",
                                     ci=128, fp=3)
            cos_sb = const.tile([128, 2, 32], bf16)
            nc.sync.dma_start(cos_sb[:], cos_d.ap().rearrange(
                "(nt p) t -> p nt t", p=128))
            sin_sb = const.tile([128, 2, 32], bf16)
            nc.sync.dma_start(sin_sb[:], sin_d.ap().rearrange(
                "(nt p) t -> p nt t", p=128))
            ident = const.tile([128, 128], bf16)
            make_identity(nc, ident)
            # pw/pb are allocated here but loaded after group 0's qkv
            # emission (queue FIFO keeps them out of the critical startup
            # bandwidth window; they are first read ~60us in)
            pw_sb = [const.tile([128, C], bf16, name=f"pw_sb{cc}")
                     for cc in range(8)]
            pb_bc = const.tile([128, C], f32)
            pw_r = pw_d.ap().rearrange("(co ci) o -> ci co o", ci=128)

            xT_r = xT_d.ap().rearrange("(co ci) n -> ci co n", ci=128)
            pending = []

            # keep the Scalar (ACT) queue clear of DMA issues — descriptor
            # generation occupies the issuing engine's FIFO and delays
            # exp/evictions queued behind it
            xfer_q = [nc.sync, nc.gpsimd]

            def load_xg(g):
                xg = [xg_p.tile([128, 512], bf16, tag=f"xg{cc}",
                                name=f"xg_{g}_{cc}") for cc in range(8)]
                for cc in range(8):
                    xfer_q[cc % 2].dma_start(
                        xg[cc][:], xT_r[:, cc, g * 512:(g + 1) * 512])
                return xg

            xg_next = None
            for g in range(4):          # group = 2 batches (512 token cols)
                if g == 0:
                    # startup-critical ordering: queues are FIFO, so land
                    # chunks in consumption order (xg chunk 0 first, then q
                    # weights (fp0), k (fp1), then v (fp2) on the gpsimd
                    # queue behind xg). pw/pb wait until after the qkv
                    # emission. Later groups' xg loads are prefetched
                    # during the previous group's attention phase.
                    xg = [xg_p.tile([128, 512], bf16, tag=f"xg{cc}",
                                    name=f"xg_0_{cc}") for cc in range(8)]
                    nc.gpsimd.dma_start(xg[0][:], xT_r[:, 0, 0:512])
                    for cc in range(1, 8):
                        xfer_q[cc % 2].dma_start(xg[cc][:],
                                                 xT_r[:, cc, 0:512])
                    for fp in range(3):
                        for cc in range(8):
                            xfer_q[cc % 2].dma_start(
                                w_sb[fp][cc][:], w_r[:, cc, fp, :])
                else:
                    xg = xg_next

                # v is stored head-interleaved with ones lanes for the
                # fused-sums PV: slot h = [v_h | ones] (even h) or
                # [ones | v_h] (odd h) so O rows and sum rows alternate
                # PSUM halves per head parity and every normalize op is
                # 64-partition aligned.
                v_g = [v_p.tile([128, 16, 128], bf16, tag=f"vg{ns}",
                                name=f"v_{g}_{ns}") for ns in range(4)]
                for ns in range(4):
                    v4 = v_g[ns][:].rearrange("p (pr q) c -> p pr q c", q=2)
                    nc.gpsimd.memset(v4[:, :, 0, 64:128], 1.0)
                    nc.gpsimd.memset(v4[:, :, 1, 0:64], 1.0)
                qkT_g = [qkT_p.tile([128, 512], bf16, tag=f"qkT{fb}",
                                    name=f"qkT_{g}_{fb}") for fb in range(16)]
                rot_tiles = {}

                # ---- qkv matmuls + rope ----
                # the two 512-wide halves of each 1024-col output share the
                # same stationary lhsT per k-chunk, letting walrus reuse the
                # loaded weights between consecutive matmuls
                for fp in range(3):     # 0: q, 1: k, 2: v (1024 cols each)
                    for ns in range(4):
                        if fp < 2:
                            raw = qkraw_p.tile([128, H, D], bf16, tag="qkraw")
                            rawf = raw[:].rearrange("p h d -> p (h d)")
                        pss = big_p.tile([128, 1024], f32, tag="big",
                                         name=f"ps_{fp}_{ns}")
                        for cc in range(8):
                            for half in range(2):
                                nc.tensor.matmul(
                                    pss[:, half * 512:(half + 1) * 512],
                                    lhsT=xg[cc][:, ns * 128:(ns + 1) * 128],
                                    rhs=w_sb[fp][cc][:, half * 512:(half + 1) * 512],
                                    start=(cc == 0), stop=(cc == 7))
                        if fp == 2:
                            # heads half*8..half*8+7: even slots keep v in
                            # cols 0:64, odd slots in cols 64:128
                            v4 = v_g[ns][:].rearrange(
                                "p (pr q) c -> p pr q c", q=2)
                            s4 = pss[:].rearrange(
                                "p (pr q c) -> p pr q c", q=2, c=64)
                            nc.any.tensor_copy(
                                out=v4[:, :, 0, 0:64], in_=s4[:, :, 0, :])
                            nc.any.tensor_copy(
                                out=v4[:, :, 1, 64:128], in_=s4[:, :, 1, :])
                        else:
                            nc.any.tensor_copy(out=rawf[:], in_=pss[:])
                        if fp == 2:
                            continue
                        # rope: evens = raw[:,:,0:32], odds = raw[:,:,32:64]
                        nt = ns % 2
                        cos = cos_sb[:, nt, None, :].to_broadcast((128, H, 32))
                        sin = sin_sb[:, nt, None, :].to_broadcast((128, H, 32))
                        qe = raw[:, :, 0:32]
                        qo = raw[:, :, 32:64]
                        t1 = tmp_p.tile([128, H, 32], bf16, tag="tmp")
                        t2 = tmp_p.tile([128, H, 32], bf16, tag="tmp")
                        nc.vector.tensor_tensor(out=t1[:], in0=qe, in1=cos, op=MUL)
                        nc.vector.tensor_tensor(out=t2[:], in0=qo, in1=sin, op=MUL)
                        rot = qkrot_p.tile([128, H, D], bf16, tag="rot")
                        nc.vector.tensor_tensor(out=rot[:, :, 0:32],
                                                in0=t1[:], in1=t2[:], op=SUB)
                        t3 = tmp_p.tile([128, H, 32], bf16, tag="tmp")
                        t4 = tmp_p.tile([128, H, 32], bf16, tag="tmp")
                        nc.vector.tensor_tensor(out=t3[:], in0=qo, in1=cos, op=MUL)
                        nc.vector.tensor_tensor(out=t4[:], in0=qe, in1=sin, op=MUL)
                        nc.vector.tensor_tensor(out=rot[:, :, 32:64],
                                                in0=t3[:], in1=t4[:], op=ADD)
                        rot_tiles[(fp, ns)] = rot

                # ---- transpose q,k into [d, n] layout (PE identity mm) ----
                # emitted in score-consumption order (q0, k0, q1, k1, ...)
                # so batch 0's first scores never wait on a late k eviction
                for fb in [b for p8 in range(8) for b in (p8, 8 + p8)]:
                    fcol = (fb % 8) * 128
                    pst = psmm_p.tile([128, 512], bf16, tag="mm",
                                      name=f"pst_{g}_{fb}")
                    for ns in range(4):
                        rot = rot_tiles[(fb // 8, ns)]
                        nc.tensor.transpose(
                            pst[:, ns * 128:(ns + 1) * 128],
                            rot[:].rearrange("p h d -> p (h d)")[:, fcol:fcol + 128],
                            ident)
                    nc.any.tensor_copy(out=qkT_g[fb][:], in_=pst[:])
                if g == 0:
                    for cc in range(8):
                        xfer_q[cc % 2].dma_start(
                            pw_sb[cc][:], pw_r[:, cc, :])
                    nc.sync.dma_start(pb_bc[:],
                                      pb_d.ap().to_broadcast((128, C)))
                # prefetch next group's x chunks so the group-boundary qkv
                # matmuls never wait on HBM
                xg_next = load_xg(g + 1) if g < 3 else None

                # ---- attention per batch ----
                # per-head pipeline: scores (PE) -> exp (ScalarE) ->
                # fused-sums PV (PE, O rows + replicated denominator in one
                # [128, 256] PSUM region) -> approx-reciprocal + normalize
                # mul (DVE, all local, no DMA). The previous batch's output
                # projection is spread through this batch's head loop in 4
                # chunks so its PSUM-accumulation tails overlap scores.
                for bb in range(2):
                    b_loc = 2 * g + bb
                    oT_b = oT_p.tile([128, 8, 256], bf16, tag="oT")
                    pTs = {}
                    # head-pair loop, software-pipelined by two pairs:
                    # scores of both heads of pair `m` land in one 2-bank
                    # PSUM tile so a single exp covers them, and the PV +
                    # normalize of pair `m-2` are emitted after pair `m`'s
                    # scores so the in-order PE never stalls scores behind
                    # a PV whose exp has not drained from the ACT queue.
                    for m in range(10):
                        if m < 8:
                            ps_s = big_p.tile([128, 4, 256], f32, tag="big")
                            for h2 in range(2):
                                h = 2 * m + h2
                                qfb, qrow = h // 2, (h % 2) * 64
                                kfb = 8 + h // 2
                                for jc in range(2):
                                    nc.tensor.matmul(
                                        ps_s[:, h2 * 2 + jc, :],
                                        lhsT=qkT_g[kfb][qrow:qrow + 64,
                                                       bb * 256 + jc * 128:bb * 256 + jc * 128 + 128],
                                        rhs=qkT_g[qfb][qrow:qrow + 64,
                                                       bb * 256:bb * 256 + 256],
                                        start=True, stop=True)
                            pT = pT_p.tile([128, 4, 256], bf16, tag="pT")
                            nc.scalar.activation(pT[:], ps_s[:], Exp)
                            pTs[m] = pT
                        if m >= 2:
                            mp = m - 2
                            pT = pTs.pop(mp)
                            ps_pair = pso_p.tile([128, 512], f32, tag="o")
                            for h2 in range(2):
                                h = 2 * mp + h2
                                po = ps_pair[:, h2 * 256:h2 * 256 + 256]
                                for jc in range(2):
                                    nc.tensor.matmul(
                                        po,
                                        lhsT=v_g[bb * 2 + jc][:].rearrange(
                                            "p s c -> p (s c)")[:, h * 128:(h + 1) * 128],
                                        rhs=pT[:, h2 * 2 + jc, :],
                                        start=(jc == 0), stop=(jc == 1))
                            # normalize the pair: gather the two heads'
                            # replicated sums (window-aligned copies, one
                            # on DVE and one on ScalarE to balance engine
                            # load; the custom-DVE approx reciprocal only
                            # works at base partition 0 full-window), one
                            # reciprocal, two muls with cross-window in1
                            # reads (verified OK on hw).
                            ssb = rsb_p.tile([128, 256], f32, tag="ssb")
                            if mp % 2 == 0:
                                nc.vector.tensor_copy(
                                    out=ssb[64:128, :],
                                    in_=ps_pair[64:128, 0:256])
                                nc.scalar.copy(out=ssb[0:64, :],
                                               in_=ps_pair[0:64, 256:512])
                            else:
                                nc.scalar.copy(out=ssb[64:128, :],
                                               in_=ps_pair[64:128, 0:256])
                                nc.vector.tensor_copy(
                                    out=ssb[0:64, :],
                                    in_=ps_pair[0:64, 256:512])
                            rsb = rsb_p.tile([128, 256], f32, tag="rsb")
                            nc.vector.reciprocal_approx_fast(rsb[:], ssb[:])
                            nc.vector.tensor_tensor(
                                out=oT_b[0:64, mp, :],
                                in0=ps_pair[0:64, 0:256],
                                in1=rsb[64:128, :], op=MUL)
                            nc.vector.tensor_tensor(
                                out=oT_b[64:128, mp, :],
                                in0=ps_pair[64:128, 256:512],
                                in1=rsb[0:64, :], op=MUL)
                        if m % 2 == 1 and pending:
                            pending.pop(0)()

                    # ---- output projection (deferred, 4 chunks) ----
                    def make_proj(b_loc, oT_b):
                        chunks = []
                        for nt2 in range(2):
                          for oc in range(2):
                            def chunk(nt2=nt2, oc=oc, b_loc=b_loc, oT_b=oT_b):
                                ps_p = psmm_p.tile([128, 2, 256], f32,
                                                   tag="mm")
                                ps_pv = ps_p[:].rearrange("p a b -> p (a b)")
                                for cc in range(8):
                                    nc.tensor.matmul(
                                        ps_pv,
                                        lhsT=oT_b[:, cc,
                                                  nt2 * 128:(nt2 + 1) * 128],
                                        rhs=pw_sb[cc][:,
                                                      oc * 512:(oc + 1) * 512],
                                        start=(cc == 0), stop=(cc == 7))
                                y_sb = y_p.tile([128, 512], f32, tag="y")
                                if with_bias:
                                    nc.vector.tensor_tensor(
                                        out=y_sb[:], in0=ps_pv,
                                        in1=pb_bc[:, oc * 512:(oc + 1) * 512],
                                        op=ADD)
                                elif oc == 0:
                                    nc.scalar.copy(out=y_sb[:], in_=ps_pv)
                                else:
                                    nc.vector.tensor_copy(out=y_sb[:],
                                                          in_=ps_pv)
                                row0 = b_loc * 256 + nt2 * 128
                                nc.sync.dma_start(
                                    out_d.ap()[row0:row0 + 128,
                                               oc * 512:(oc + 1) * 512],
                                    y_sb[:])
                            chunks.append(chunk)
                        return chunks
                    pending.extend(make_proj(b_loc, oT_b))

            while pending:
                pending.pop(0)()

    nc.compile()
    _cache[key] = nc
    return nc


def _prep_inputs(x, qkv_w, proj_w, proj_b):
    perm = np.concatenate([np.arange(0, D, 2), np.arange(1, D, 2)])  # evens|odds
    head_perm = (np.arange(H)[:, None] * D + perm[None, :]).reshape(-1)
    wq = qkv_w[:C][head_perm] * np.float32(D ** -0.5)
    wk = qkv_w[C:2 * C][head_perm]
    wv = qkv_w[2 * C:]
    wT = np.ascontiguousarray(
        np.concatenate([wq, wk, wv], 0).T).astype(ml_dtypes.bfloat16)
    pwT = np.ascontiguousarray(proj_w.T).astype(ml_dtypes.bfloat16)
    pb = np.ascontiguousarray(proj_b.reshape(1, C)).astype(np.float32)
    cosH, sinH = _rope_tables()
    cosH = cosH.astype(ml_dtypes.bfloat16)
    sinH = sinH.astype(ml_dtypes.bfloat16)

    in_maps = []
    for c in range(N_CORES):
        xs = x[c * B_LOC:(c + 1) * B_LOC].reshape(NT, C)
        xT = np.ascontiguousarray(xs.T).astype(ml_dtypes.bfloat16)
        in_maps.append({"xT": xT, "wT": wT, "pwT": pwT, "pb": pb,
                        "cosH": cosH, "sinH": sinH})
    return in_maps


def _run(inputs, trace=False, **kw):
    nc = _build(with_bias=bool(np.any(inputs["proj_b"])))
    in_maps = _prep_inputs(inputs["x"], inputs["qkv_w"],
                           inputs["proj_w"], inputs["proj_b"])
    res = run_bass_kernel_spmd(nc, in_maps, core_ids=list(range(N_CORES)),
                               trace=trace, **kw)
    out = np.concatenate([res.results[c]["out"] for c in range(N_CORES)], 0)
    return out.reshape(B, N, C).astype(np.float32), res


def kernel(x, qkv_w, proj_w, proj_b):
    x = np.asarray(x, dtype=np.float32)
    qkv_w = np.asarray(qkv_w, dtype=np.float32)
    proj_w = np.asarray(proj_w, dtype=np.float32)
    proj_b = np.asarray(proj_b, dtype=np.float32)
    out, _ = _run({"x": x, "qkv_w": qkv_w, "proj_w": proj_w,
                   "proj_b": proj_b})
    return out

